# revision 10
# baseline (speedup 1.0000x reference)
"""Channelwise symmetric Hausdorff distance loss on 8 Trainium2 NeuronCores.

Math (per (batch, channel) pair; x, y are [N, D] point sets):
    d2[n, m] = |x_n|^2 + |y_m|^2 - 2 x_n.y_m
    h = max( max_n min_m d(n,m), max_m min_n d(n,m) )
    answer   = mean over the B*C pairs of h.

Sharding: B*C = 24 pairs, 3 per NeuronCore (data parallel), host gathers.

Per-core device kernel (v4):
  - host-prepped inputs, partition-major so each DMA moves 2-4 KB contiguous
    per partition (large packets stream ~4x faster per queue than 1 KB ones):
      xtp = (-2 x)^T fp8 [128, KT*N] (stationary side, chunk-major per part.)
      ytp = y^T      fp8 [128, KT*N] (moving side)
      fold fp16 [4, 2N]: cols 0..N-1   = [1; 1; x2_hi; x2_lo] over n,
                         cols N..2N-1  = [y2_hi; y2_lo; 1; 1] over m.
  - ALL input DMAs trigger up-front on the SYNC ring only, in priority
    order (pair 0 quarters first): per-ring FIFO means pair 0's data is not
    slowed down by pair 1/2 prefetches (separate rings race in parallel).
  - ~10 warm-up matmuls on scratch data run while the first DMAs land,
    flipping the PE HAM clock gate from 1.2 to 2.4 GHz before real work.
  - per n-tile:
      1. K=4 fp16 fold matmul per m-block (start=True):
           psum = fold_x[:, n]^T @ fold_y[:, m] = x2[n] + y2[m]
         (one matmul streams BOTH bias terms; same cost as a y2-only fold).
      2. 8 accumulating fp8 DoubleRow matmuls: psum += -2 x.y -> full d2.
         (measured: back-to-back 512-col DoubleRow matmuls issue every
         ~216ns; LDWEIGHTS overlaps.)
      3. scalar engine copies psum out as bf16 d2p [128, N] (1114ns; the
         only psum read - DVE ops then run from SBUF bf16 at 2x/lane).
      4. DVE: h1 = min(d2p[:, :512], d2p[:, 512:])      (TT bf16, 405ns)
              rowaccs[:, idx] = min_m h1               (reduce 512, 672ns)
              colacc = min(colacc, d2p)                (TT bf16, 692ns)
         (fused scalar_tensor_tensor / full-width reduce measured 1.9-2.6x
         slower - they have no packed-bf16 uops on this DVE table.)
  - outputs: rowaccs [128, PP*NT] bf16 (min_m d2), colacc [128, N] bf16.
Host finishes in float64: fwd2 = max(rowaccs), bwd2 = max_m(min_p colacc),
h = sqrt(max(fwd2, bwd2, 0)), mean over 24 pairs.
"""

import numpy as np

B, C, N, D = 8, 3, 1024, 1024
N_CORES = 8
PAIRS = B * C              # 24
PP = PAIRS // N_CORES      # 3 pairs per core
NT = N // 128              # 8 n-tiles (output partition dim)
MBS = 512                  # m block size (one PSUM bank of fp32)
MB = N // MBS              # 2 m-blocks
KT = D // 128              # 8 k-tiles (contraction)
QRT = KT * N // 4          # fp8 bytes per partition per quarter-tensor DMA
N_WARMUP = 10              # warm-up matmuls (~4us cold -> HAM 8/8)

_NC_CACHE = None


def _legalize_sync(nc):
    """This toolchain's walrus accepts at most ONE sync-wait per instruction;
    Tile emits several (e.g. the tail drain waits on every engine/DMA sem).
    Hoist all but the last wait of each instruction into standalone
    InstEventSemaphore instructions on the same engine, inserted just before
    it — semantically identical (the engine blocks on each in turn)."""
    import concourse.mybir as mybir

    n_split = 0
    for fn in nc.m.functions:
        for bb in fn.blocks:
            new_il = []
            for ins in bb.instructions:
                si = ins.sync_info
                if si is not None and si.on_wait and len(si.on_wait) > 1:
                    waits = list(si.on_wait)
                    for k, w in enumerate(waits[:-1]):
                        ev = mybir.InstEventSemaphore(
                            name=f"{ins.name}-evw{k}",
                            engine=ins.engine,
                            ins=[],
                            outs=[],
                            sync_info=mybir.SyncInfo(on_wait=[w], on_update=[]),
                        )
                        new_il.append(ev)
                        n_split += 1
                    si.on_wait = [waits[-1]]
                new_il.append(ins)
            bb.instructions[:] = new_il
    return n_split


def _build_nc():
    import concourse.bass as bass
    import concourse.mybir as mybir
    import concourse.tile as tile

    bf16 = mybir.dt.bfloat16
    f16 = mybir.dt.float16
    f32 = mybir.dt.float32
    f8 = mybir.dt.float8e4
    op_add = mybir.AluOpType.add
    op_min = mybir.AluOpType.min

    nc = bass.Bass("TRN2", target_bir_lowering=True, debug=False)
    xt_d = nc.dram_tensor("xtp", [PP, 128, KT * N], f8, kind="ExternalInput").ap()
    yt_d = nc.dram_tensor("ytp", [PP, 128, KT * N], f8, kind="ExternalInput").ap()
    fold_d = nc.dram_tensor("fold", [PP, 4, 2 * N], f16, kind="ExternalInput").ap()
    row_d = nc.dram_tensor("rowout", [128, PP * NT], bf16, kind="ExternalOutput").ap()
    col_d = nc.dram_tensor("colout", [PP, 128, N], bf16, kind="ExternalOutput").ap()

    with tile.TileContext(nc) as tc:
        with (
            tc.tile_pool(name="const", bufs=1) as const_pool,
            tc.tile_pool(name="xy", bufs=3) as xy_pool,
            tc.tile_pool(name="d2", bufs=3) as d2_pool,
            tc.tile_pool(name="h1", bufs=2) as h1_pool,
            tc.tile_pool(name="col", bufs=2) as col_pool,
            tc.tile_pool(name="ps", bufs=4, space="PSUM") as ps_pool,
        ):
            ones2 = const_pool.tile([2, 128], f16)
            nc.vector.memset(ones2, 1.0)
            wu_mov = const_pool.tile([2, MBS], f16)
            nc.vector.memset(wu_mov, 1.0)
            rowaccs = const_pool.tile([128, PP * NT], bf16)

            # ---- input DMAs: all on the sync ring, priority order ----
            xt_sb = [
                xy_pool.tile([128, KT * N], f8, tag="xt", name=f"xt{j}")
                for j in range(PP)
            ]
            yt_sb = [
                xy_pool.tile([128, KT * N], f8, tag="yt", name=f"yt{j}")
                for j in range(PP)
            ]
            fold_sb = const_pool.tile([4, PP * 2 * N], f16)
            # pair 0: interleaved quarters so matmuls can chase the data
            for q in range(4):
                sl = slice(q * QRT, (q + 1) * QRT)
                nc.sync.dma_start(out=xt_sb[0][:, sl], in_=xt_d[0, :, sl])
                nc.sync.dma_start(out=yt_sb[0][:, sl], in_=yt_d[0, :, sl])
            nc.sync.dma_start(
                out=fold_sb.rearrange("p (j m) -> p j m", j=PP), in_=fold_d
            )
            for j in (1, 2):
                for hq in range(2):
                    sl = slice(hq * 2 * QRT, (hq + 1) * 2 * QRT)
                    nc.sync.dma_start(out=xt_sb[j][:, sl], in_=xt_d[j, :, sl])
                    nc.sync.dma_start(out=yt_sb[j][:, sl], in_=yt_d[j, :, sl])

            # ---- PE warm-up: flip HAM to 8/8 while DMAs land ----
            ps_wu = ps_pool.tile([128, MB, MBS], f32, tag="ps")
            for i in range(N_WARMUP):
                nc.tensor.matmul(
                    ps_wu[:, i % MB, :], ones2, wu_mov, start=True, stop=True
                )

            for j in range(PP):
                xt3 = xt_sb[j].rearrange("p (k n) -> p k n", k=KT)
                yt3 = yt_sb[j].rearrange("p (k n) -> p k n", k=KT)
                fx = fold_sb[:, j * 2 * N : j * 2 * N + N]        # [4, N] over n
                fy = fold_sb[:, j * 2 * N + N : (j + 1) * 2 * N]  # [4, N] over m
                colacc = col_pool.tile([128, N], bf16, tag="colacc", name=f"col{j}")

                for nt in range(NT):
                    nsl = slice(nt * 128, (nt + 1) * 128)
                    ps = ps_pool.tile([128, MB, MBS], f32, tag="ps")
                    # bias fold: psum = x2[n] + y2[m]   (K=4 fp16 matmul)
                    for mb in range(MB):
                        nc.tensor.matmul(
                            ps[:, mb, :],
                            fx[:, nsl],
                            fy[:, mb * MBS : (mb + 1) * MBS],
                            start=True,
                            stop=False,
                        )
                    # data: psum += -2 x.y   (fp8 DoubleRow)
                    for ki in range(KT // 2):
                        xsl = xt3[:, 2 * ki : 2 * ki + 2, nsl]
                        for mb in range(MB):
                            nc.tensor.matmul(
                                ps[:, mb, :],
                                xsl,
                                yt3[:, 2 * ki : 2 * ki + 2, mb * MBS : (mb + 1) * MBS],
                                start=False,
                                stop=(ki == KT // 2 - 1 and mb == MB - 1),
                                perf_mode=mybir.MatmulPerfMode.DoubleRow,
                            )
                    # the only psum read: d2p = psum (full d2) as bf16
                    d2p = d2_pool.tile([128, N], bf16, tag="d2p")
                    nc.scalar.copy(
                        out=d2p.rearrange("p (a m) -> p a m", a=MB), in_=ps
                    )
                    idx = j * NT + nt
                    # rowaccs[:, idx] = min_m d2p  (TT-half + 512-reduce)
                    h1 = h1_pool.tile([128, MBS], bf16, tag="h1")
                    nc.vector.tensor_tensor(
                        out=h1, in0=d2p[:, :MBS], in1=d2p[:, MBS:], op=op_min
                    )
                    nc.vector.tensor_reduce(
                        out=rowaccs[:, idx : idx + 1],
                        in_=h1,
                        axis=mybir.AxisListType.X,
                        op=op_min,
                    )
                    # colacc = min(colacc, d2p)
                    if nt == 0:
                        nc.vector.tensor_scalar(
                            out=colacc,
                            in0=d2p,
                            scalar1=0.0,
                            scalar2=None,
                            op0=op_add,
                        )
                    else:
                        nc.vector.tensor_tensor(
                            out=colacc, in0=d2p, in1=colacc, op=op_min
                        )
                nc.sync.dma_start(out=col_d[j], in_=colacc)
            nc.sync.dma_start(out=row_d, in_=rowaccs)
    _legalize_sync(nc)
    return nc


def _prep_inputs(x, y):
    import ml_dtypes

    f8np = np.dtype(ml_dtypes.float8_e4m3)
    x32 = np.ascontiguousarray(x, dtype=np.float32).reshape(PAIRS, N, D)
    y32 = np.ascontiguousarray(y, dtype=np.float32).reshape(PAIRS, N, D)

    # xtp[q, p, k*N + n] = -2 x[q, n, k*128 + p]; ytp[q, p, k*N+m] = y[q,m,k*128+p]
    xtp = np.empty((PAIRS, 128, KT * N), f8np)
    ytp = np.empty((PAIRS, 128, KT * N), f8np)
    for q in range(PAIRS):
        xt = (x32[q].T * np.float32(-2.0)).astype(f8np)   # [D, N]
        yt = y32[q].T.astype(f8np)
        xtp[q] = xt.reshape(KT, 128, N).transpose(1, 0, 2).reshape(128, KT * N)
        ytp[q] = yt.reshape(KT, 128, N).transpose(1, 0, 2).reshape(128, KT * N)

    x2 = np.square(x32.astype(np.float64)).sum(-1)  # [PAIRS, N]
    y2 = np.square(y32.astype(np.float64)).sum(-1)
    # fold[q]: [4, 2N] fp16; cols 0..N-1: [1;1;x2hi;x2lo], cols N..: [y2hi;y2lo;1;1]
    fold = np.zeros((PAIRS, 4, 2 * N), np.float16)
    x2_hi = x2.astype(np.float16)
    x2_lo = (x2 - x2_hi.astype(np.float64)).astype(np.float16)
    y2_hi = y2.astype(np.float16)
    y2_lo = (y2 - y2_hi.astype(np.float64)).astype(np.float16)
    fold[:, 0, :N] = 1.0
    fold[:, 1, :N] = 1.0
    fold[:, 2, :N] = x2_hi
    fold[:, 3, :N] = x2_lo
    fold[:, 0, N:] = y2_hi
    fold[:, 1, N:] = y2_lo
    fold[:, 2, N:] = 1.0
    fold[:, 3, N:] = 1.0
    return xtp, ytp, fold


def _run(x, y, trace=False):
    global _NC_CACHE
    from concourse.bass_utils import run_bass_kernel_spmd

    xtp, ytp, fold = _prep_inputs(x, y)

    if _NC_CACHE is None:
        _NC_CACHE = _build_nc()
    nc = _NC_CACHE

    in_maps = []
    for i in range(N_CORES):
        q0 = i * PP
        in_maps.append(
            {
                "xtp": xtp[q0 : q0 + PP],
                "ytp": ytp[q0 : q0 + PP],
                "fold": fold[q0 : q0 + PP],
            }
        )

    res = run_bass_kernel_spmd(nc, in_maps, core_ids=list(range(N_CORES)), trace=trace)

    h2 = np.empty(PAIRS, np.float64)
    for i in range(N_CORES):
        r = res.results[i]
        row = r["rowout"].astype(np.float64)  # [128, PP*NT], = min_m d2
        for j in range(PP):
            fwd2 = row[:, j * NT : (j + 1) * NT].max()
            # colacc[p, m] = min over n-tiles of full d2 in bf16
            cmin = r["colout"][j].astype(np.float64).min(0)  # [N]
            bwd2 = cmin.max()
            h2[i * PP + j] = max(fwd2, bwd2, 0.0)

    ans = np.sqrt(h2).mean()
    return np.array(ans, dtype=np.float32), res


def kernel(input, target):
    out, _ = _run(np.asarray(input), np.asarray(target), trace=False)
    return out


# revision 11
# speedup vs baseline: 1.3830x; 1.3830x over previous
"""Channelwise symmetric Hausdorff distance loss on 8 Trainium2 NeuronCores.

Math (per (batch, channel) pair; x, y are [N, D] point sets):
    d2[n, m] = |x_n|^2 + |y_m|^2 - 2 x_n.y_m
    h = max( max_n min_m d(n,m), max_m min_n d(n,m) )
    answer   = mean over the B*C pairs of h.

Sharding: B*C = 24 pairs, 3 per NeuronCore (data parallel), host gathers.

Per-core device kernel (v5) — exp-transform structure:
  Writing E = exp((ref - d2)/T), the two Hausdorff reductions become
    row side:  min_m d2[n,:]  ~=  ref - T ln(sum_m E[n,:])   (smooth-min,
               error <= T*ln(#near-min) ~ 15-20 on d2 ~ 1500, rel ~3e-3)
    col side:  min_n d2[:,m]   =  ref - T ln(max_n E[:,m])   (EXACT: exp
               is monotone, max commutes)
  which maps perfectly onto the engines:
  - PE: 8 fp8 DoubleRow matmuls per n-tile (psum += -2 x.y), start=False
    onto a y2bc preload. Pure-DoubleRow streams issue every 216ns
    (measured); any interleaved fp16 fold matmul degrades the stream, so
    the y2 bias is PRELOADED by ACT/DVE instead (engine writes to PSUM set
    has_written; accumulating matmuls add on top — verified on HW).
  - ACT (scalar engine): ONE op per n-tile does everything else on the
    row side: E = Exp(psum * (-1/T) + (ref - x2[n])/T) -> SBUF bf16, with
    fused accum_out = sum_m E  ->  rowsums[:, idx].  (per-partition bias
    AP carries x2; verified on HW.)
  - DVE: colacc = max(colacc, E) (bf16 packed tensor_tensor, 692ns) +
    a share of the y2bc preloads.
  - host-prepped inputs partition-major (2-4 KB contiguous per partition
    per DMA); ALL input DMAs trigger up-front on the SYNC ring in
    priority order (per-ring FIFO => pair 0 is not slowed by prefetches).
  - ~10 warm-up matmuls + a warm-up Exp flip the PE HAM clock gate to
    2.4 GHz and pull in the ACT exp-table load while the first DMAs land.
  - outputs: rowsums [128, PP*NT] fp32, colacc(E) [128, N] bf16 per pair.
Host finishes in float64:
    fwd2 = ref - T ln(min rowsums), bwd2 = ref - T ln(min_m max_p colacc),
    h = sqrt(max(fwd2, bwd2, 0)), mean over 24 pairs.
"""

import numpy as np

B, C, N, D = 8, 3, 1024, 1024
N_CORES = 8
PAIRS = B * C              # 24
PP = PAIRS // N_CORES      # 3 pairs per core
NT = N // 128              # 8 n-tiles (output partition dim)
MBS = 512                  # m block size (one PSUM bank of fp32)
MB = N // MBS              # 2 m-blocks
KT = D // 128              # 8 k-tiles (contraction)
QRT = KT * N // 4          # fp8 bytes per partition per quarter-tensor DMA
N_WARMUP = 10              # warm-up matmuls (~4us cold -> HAM 8/8)
T_SMOOTH = 24.0            # smooth-min temperature
REF = 1200.0               # d2 reference shift (d2 in ~[1380, 2720])

_NC_CACHE = None


def _legalize_sync(nc):
    """This toolchain's walrus accepts at most ONE sync-wait per instruction;
    Tile emits several (e.g. the tail drain waits on every engine/DMA sem).
    Hoist all but the last wait of each instruction into standalone
    InstEventSemaphore instructions on the same engine, inserted just before
    it — semantically identical (the engine blocks on each in turn)."""
    import concourse.mybir as mybir

    n_split = 0
    for fn in nc.m.functions:
        for bb in fn.blocks:
            new_il = []
            for ins in bb.instructions:
                si = ins.sync_info
                if si is not None and si.on_wait and len(si.on_wait) > 1:
                    waits = list(si.on_wait)
                    for k, w in enumerate(waits[:-1]):
                        ev = mybir.InstEventSemaphore(
                            name=f"{ins.name}-evw{k}",
                            engine=ins.engine,
                            ins=[],
                            outs=[],
                            sync_info=mybir.SyncInfo(on_wait=[w], on_update=[]),
                        )
                        new_il.append(ev)
                        n_split += 1
                    si.on_wait = [waits[-1]]
                new_il.append(ins)
            bb.instructions[:] = new_il
    return n_split


def _build_nc():
    import concourse.bass as bass
    import concourse.mybir as mybir
    import concourse.tile as tile

    bf16 = mybir.dt.bfloat16
    f16 = mybir.dt.float16
    f32 = mybir.dt.float32
    f8 = mybir.dt.float8e4
    op_add = mybir.AluOpType.add
    op_max = mybir.AluOpType.max
    EXP = mybir.ActivationFunctionType.Exp

    nc = bass.Bass("TRN2", target_bir_lowering=True, debug=False)
    xt_d = nc.dram_tensor("xtp", [PP, 128, KT * N], f8, kind="ExternalInput").ap()
    yt_d = nc.dram_tensor("ytp", [PP, 128, KT * N], f8, kind="ExternalInput").ap()
    ybc_d = nc.dram_tensor("y2bc", [PP, 128, N], f16, kind="ExternalInput").ap()
    bias_d = nc.dram_tensor("biasT", [128, PP * NT], f32, kind="ExternalInput").ap()
    row_d = nc.dram_tensor("rowsums", [128, PP * NT], f32, kind="ExternalOutput").ap()
    col_d = nc.dram_tensor("colout", [PP, 128, N], bf16, kind="ExternalOutput").ap()

    with tile.TileContext(nc) as tc:
        with (
            tc.tile_pool(name="const", bufs=1) as const_pool,
            tc.tile_pool(name="xy", bufs=3) as xy_pool,
            tc.tile_pool(name="bc", bufs=3) as bc_pool,
            tc.tile_pool(name="d2", bufs=3) as d2_pool,
            tc.tile_pool(name="col", bufs=2) as col_pool,
            tc.tile_pool(name="ps", bufs=4, space="PSUM") as ps_pool,
        ):
            ones2 = const_pool.tile([2, 128], f16)
            nc.vector.memset(ones2, 1.0)
            wu_mov = const_pool.tile([2, MBS], f16)
            nc.vector.memset(wu_mov, 1.0)
            bias_sb = const_pool.tile([128, PP * NT], f32)
            rowsums = const_pool.tile([128, PP * NT], f32)
            wu_exp = const_pool.tile([128, 16], bf16)

            # ---- input DMAs: all on the sync ring, priority order ----
            xt_sb = [
                xy_pool.tile([128, KT * N], f8, tag="xt", name=f"xt{j}")
                for j in range(PP)
            ]
            yt_sb = [
                xy_pool.tile([128, KT * N], f8, tag="yt", name=f"yt{j}")
                for j in range(PP)
            ]
            ybc_sb = [
                bc_pool.tile([128, N], f16, tag="ybc", name=f"ybc{j}")
                for j in range(PP)
            ]
            nc.sync.dma_start(out=ybc_sb[0], in_=ybc_d[0])
            for q in range(4):
                sl = slice(q * QRT, (q + 1) * QRT)
                nc.sync.dma_start(out=xt_sb[0][:, sl], in_=xt_d[0, :, sl])
                nc.sync.dma_start(out=yt_sb[0][:, sl], in_=yt_d[0, :, sl])
            nc.sync.dma_start(out=bias_sb, in_=bias_d)
            for j in (1, 2):
                nc.sync.dma_start(out=ybc_sb[j], in_=ybc_d[j])
                for hq in range(2):
                    sl = slice(hq * 2 * QRT, (hq + 1) * 2 * QRT)
                    nc.sync.dma_start(out=xt_sb[j][:, sl], in_=xt_d[j, :, sl])
                    nc.sync.dma_start(out=yt_sb[j][:, sl], in_=yt_d[j, :, sl])

            # ---- warm-ups while DMAs land: PE HAM -> 8/8, ACT exp table ----
            nc.scalar.activation(
                out=wu_exp, in_=wu_exp, func=EXP, bias=0.0, scale=-1.0
            )
            ps_wu = ps_pool.tile([128, MB, MBS], f32, tag="ps")
            for i in range(N_WARMUP):
                nc.tensor.matmul(
                    ps_wu[:, i % MB, :], ones2, wu_mov, start=True, stop=True
                )

            units = [(j, nt) for j in range(PP) for nt in range(NT)]
            colaccs = [None] * PP

            def preload(u):
                ps_u = ps_pool.tile([128, MB, MBS], f32, tag="ps", name=f"ps{u}")
                jv = units[u][0]
                src = ybc_sb[jv].rearrange("p (a m) -> p a m", a=MB)
                if u % 3 == 0:
                    nc.scalar.copy(out=ps_u, in_=src)
                else:
                    nc.vector.tensor_scalar(
                        out=ps_u, in0=src, scalar1=0.0, scalar2=None, op0=op_add
                    )
                return ps_u

            ps_next = preload(0)
            for u, (j, nt) in enumerate(units):
                ps = ps_next
                xt3 = xt_sb[j].rearrange("p (k n) -> p k n", k=KT)
                yt3 = yt_sb[j].rearrange("p (k n) -> p k n", k=KT)
                nsl = slice(nt * 128, (nt + 1) * 128)
                for ki in range(KT // 2):
                    xsl = xt3[:, 2 * ki : 2 * ki + 2, nsl]
                    for mb in range(MB):
                        nc.tensor.matmul(
                            ps[:, mb, :],
                            xsl,
                            yt3[:, 2 * ki : 2 * ki + 2, mb * MBS : (mb + 1) * MBS],
                            start=False,  # accumulate onto the y2bc preload
                            stop=(ki == KT // 2 - 1 and mb == MB - 1),
                            perf_mode=mybir.MatmulPerfMode.DoubleRow,
                        )
                # preload the NEXT tile before this one's copy-out so the
                # preloading engine stays a tile ahead of the PE
                if u + 1 < len(units):
                    ps_next = preload(u + 1)
                idx = j * NT + nt
                # ONE ACT op: E = exp((ref - x2[n] - psum)/T) -> bf16 SBUF,
                # fused rowsums[:, idx] = sum_m E
                ee = d2_pool.tile([128, N], bf16, tag="ee")
                nc.scalar.activation(
                    out=ee.rearrange("p (a m) -> p a m", a=MB),
                    in_=ps,
                    func=EXP,
                    bias=bias_sb[:, idx : idx + 1],
                    scale=-1.0 / T_SMOOTH,
                    accum_out=rowsums[:, idx : idx + 1],
                )
                # colacc = max(colacc, E)   (bf16 packed TT)
                if nt == 0:
                    colaccs[j] = col_pool.tile(
                        [128, N], bf16, tag="colacc", name=f"col{j}"
                    )
                    nc.vector.tensor_scalar(
                        out=colaccs[j],
                        in0=ee,
                        scalar1=0.0,
                        scalar2=None,
                        op0=op_add,
                    )
                else:
                    nc.vector.tensor_tensor(
                        out=colaccs[j], in0=ee, in1=colaccs[j], op=op_max
                    )
                if nt == NT - 1:
                    nc.sync.dma_start(out=col_d[j], in_=colaccs[j])
            nc.sync.dma_start(out=row_d, in_=rowsums)
    _legalize_sync(nc)
    return nc


def _prep_inputs(x, y):
    import ml_dtypes

    f8np = np.dtype(ml_dtypes.float8_e4m3)
    x32 = np.ascontiguousarray(x, dtype=np.float32).reshape(PAIRS, N, D)
    y32 = np.ascontiguousarray(y, dtype=np.float32).reshape(PAIRS, N, D)

    # xtp[q, p, k*N + n] = -2 x[q, n, k*128 + p]; ytp[q, p, k*N+m] = y[q,m,k*128+p]
    xtp = np.empty((PAIRS, 128, KT * N), f8np)
    ytp = np.empty((PAIRS, 128, KT * N), f8np)
    for q in range(PAIRS):
        xt = (x32[q].T * np.float32(-2.0)).astype(f8np)   # [D, N]
        yt = y32[q].T.astype(f8np)
        xtp[q] = xt.reshape(KT, 128, N).transpose(1, 0, 2).reshape(128, KT * N)
        ytp[q] = yt.reshape(KT, 128, N).transpose(1, 0, 2).reshape(128, KT * N)

    x2 = np.square(x32.astype(np.float64)).sum(-1)  # [PAIRS, N]
    y2 = np.square(y32.astype(np.float64)).sum(-1)
    # y2 broadcast to all partitions, fp16 (quantization +-0.5 at ~2048)
    y2bc = np.ascontiguousarray(
        np.broadcast_to(y2.astype(np.float16)[:, None, :], (PAIRS, 128, N))
    )
    # biasT[q, p, t] = (REF - x2[q, t*128+p]) / T
    biasT = np.ascontiguousarray(
        ((REF - x2) / T_SMOOTH)
        .reshape(PAIRS, NT, 128)
        .transpose(0, 2, 1)
        .astype(np.float32)
    )
    return xtp, ytp, y2bc, biasT


def _run(x, y, trace=False):
    global _NC_CACHE
    from concourse.bass_utils import run_bass_kernel_spmd

    xtp, ytp, y2bc, biasT = _prep_inputs(x, y)

    if _NC_CACHE is None:
        _NC_CACHE = _build_nc()
    nc = _NC_CACHE

    in_maps = []
    for i in range(N_CORES):
        q0 = i * PP
        bias_core = np.ascontiguousarray(
            biasT[q0 : q0 + PP].transpose(1, 0, 2).reshape(128, PP * NT)
        )
        in_maps.append(
            {
                "xtp": xtp[q0 : q0 + PP],
                "ytp": ytp[q0 : q0 + PP],
                "y2bc": y2bc[q0 : q0 + PP],
                "biasT": bias_core,
            }
        )

    res = run_bass_kernel_spmd(nc, in_maps, core_ids=list(range(N_CORES)), trace=trace)

    TINY = 1e-300
    h2 = np.empty(PAIRS, np.float64)
    for i in range(N_CORES):
        r = res.results[i]
        rs = r["rowsums"].astype(np.float64)  # [128, PP*NT]
        for j in range(PP):
            # fwd2 = max_n (ref - T ln rowsum[n]) = ref - T ln(min rowsum)
            rmin = max(rs[:, j * NT : (j + 1) * NT].min(), TINY)
            fwd2 = REF - T_SMOOTH * np.log(rmin)
            # bwd2 = max_m (ref - T ln(max_p colacc[p, m]))
            cmax = np.maximum(
                r["colout"][j].astype(np.float64).max(0), TINY
            )  # [N]
            bwd2 = REF - T_SMOOTH * np.log(cmax.min())
            h2[i * PP + j] = max(fwd2, bwd2, 0.0)

    ans = np.sqrt(h2).mean()
    return np.array(ans, dtype=np.float32), res


def kernel(input, target):
    out, _ = _run(np.asarray(input), np.asarray(target), trace=False)
    return out


# revision 14
# speedup vs baseline: 1.4809x; 1.0708x over previous
"""Channelwise symmetric Hausdorff distance loss on 8 Trainium2 NeuronCores.

Math (per (batch, channel) pair; x, y are [N, D] point sets):
    d2[n, m] = |x_n|^2 + |y_m|^2 - 2 x_n.y_m
    h = max( max_n min_m d(n,m), max_m min_n d(n,m) )
    answer   = mean over the B*C pairs of h.

Sharding: B*C = 24 pairs, 3 per NeuronCore (data parallel), host gathers.

Per-core device kernel (v5) — exp-transform structure:
  Writing E = exp((ref - d2)/T), the two Hausdorff reductions become
    row side:  min_m d2[n,:]  ~=  ref - T ln(sum_m E[n,:])   (smooth-min,
               error <= T*ln(#near-min) ~ 15-20 on d2 ~ 1500, rel ~3e-3)
    col side:  min_n d2[:,m]   =  ref - T ln(max_n E[:,m])   (EXACT: exp
               is monotone, max commutes)
  which maps perfectly onto the engines:
  - PE: 8 fp8 DoubleRow matmuls per n-tile (psum += -2 x.y), start=False
    onto a y2bc preload. Pure-DoubleRow streams issue every 216ns
    (measured); any interleaved fp16 fold matmul degrades the stream, so
    the y2 bias is PRELOADED by ACT/DVE instead (engine writes to PSUM set
    has_written; accumulating matmuls add on top — verified on HW).
  - ACT (scalar engine): ONE op per n-tile does everything else on the
    row side: E = Exp(psum * (-1/T) + (ref - x2[n])/T) -> SBUF bf16, with
    fused accum_out = sum_m E  ->  rowsums[:, idx].  (per-partition bias
    AP carries x2; verified on HW.)
  - DVE: colacc = max(colacc, E) (bf16 packed tensor_tensor, 692ns) +
    a share of the y2bc preloads.
  - host-prepped inputs partition-major (2-4 KB contiguous per partition
    per DMA); ALL input DMAs trigger up-front on the SYNC ring in
    priority order (per-ring FIFO => pair 0 is not slowed by prefetches).
  - ~10 warm-up matmuls + a warm-up Exp flip the PE HAM clock gate to
    2.4 GHz and pull in the ACT exp-table load while the first DMAs land.
  - outputs: rowsums [128, PP*NT] fp32, colacc(E) [128, N] bf16 per pair.
Host finishes in float64:
    fwd2 = ref - T ln(min rowsums), bwd2 = ref - T ln(min_m max_p colacc),
    h = sqrt(max(fwd2, bwd2, 0)), mean over 24 pairs.
"""

import numpy as np

B, C, N, D = 8, 3, 1024, 1024
N_CORES = 8
PAIRS = B * C              # 24
PP = PAIRS // N_CORES      # 3 pairs per core
NT = N // 128              # 8 n-tiles (output partition dim)
MBS = 512                  # m block size (one PSUM bank of fp32)
MB = N // MBS              # 2 m-blocks
KT = D // 128              # 8 k-tiles (contraction)
QRT = KT * N // 4          # fp8 bytes per partition per quarter-tensor DMA
N_WARMUP = 10              # warm-up matmuls (~4us cold -> HAM 8/8)
T_SMOOTH = 24.0            # smooth-min temperature
REF = 1200.0               # d2 reference shift (d2 in ~[1380, 2720])

_NC_CACHE = None


def _legalize_sync(nc):
    """This toolchain's walrus accepts at most ONE sync-wait per instruction;
    Tile emits several (e.g. the tail drain waits on every engine/DMA sem).
    Hoist all but the last wait of each instruction into standalone
    InstEventSemaphore instructions on the same engine, inserted just before
    it — semantically identical (the engine blocks on each in turn)."""
    import concourse.mybir as mybir

    n_split = 0
    for fn in nc.m.functions:
        for bb in fn.blocks:
            new_il = []
            for ins in bb.instructions:
                si = ins.sync_info
                if si is not None and si.on_wait and len(si.on_wait) > 1:
                    waits = list(si.on_wait)
                    for k, w in enumerate(waits[:-1]):
                        ev = mybir.InstEventSemaphore(
                            name=f"{ins.name}-evw{k}",
                            engine=ins.engine,
                            ins=[],
                            outs=[],
                            sync_info=mybir.SyncInfo(on_wait=[w], on_update=[]),
                        )
                        new_il.append(ev)
                        n_split += 1
                    si.on_wait = [waits[-1]]
                new_il.append(ins)
            bb.instructions[:] = new_il
    return n_split


def _build_nc():
    import concourse.bass as bass
    import concourse.mybir as mybir
    import concourse.tile as tile

    bf16 = mybir.dt.bfloat16
    f16 = mybir.dt.float16
    f32 = mybir.dt.float32
    f8 = mybir.dt.float8e4
    op_add = mybir.AluOpType.add
    op_max = mybir.AluOpType.max
    EXP = mybir.ActivationFunctionType.Exp

    nc = bass.Bass("TRN2", target_bir_lowering=True, debug=False)
    xt_d = nc.dram_tensor("xtp", [PP, 128, KT * N], f8, kind="ExternalInput").ap()
    yt_d = nc.dram_tensor("ytp", [PP, 128, KT * N], f8, kind="ExternalInput").ap()
    ybc_d = nc.dram_tensor("y2bc", [PP, 128, N], f16, kind="ExternalInput").ap()
    bias_d = nc.dram_tensor("biasT", [128, PP * NT], f32, kind="ExternalInput").ap()
    row_d = nc.dram_tensor("rowsums", [128, PP * NT], f32, kind="ExternalOutput").ap()
    col_d = nc.dram_tensor("colout", [PP, 128, N], bf16, kind="ExternalOutput").ap()

    with tile.TileContext(nc) as tc:
        with (
            tc.tile_pool(name="const", bufs=1) as const_pool,
            tc.tile_pool(name="xy", bufs=3) as xy_pool,
            tc.tile_pool(name="bc", bufs=3) as bc_pool,
            tc.tile_pool(name="d2", bufs=3) as d2_pool,
            tc.tile_pool(name="col", bufs=2) as col_pool,
            tc.tile_pool(name="ps", bufs=4, space="PSUM") as ps_pool,
        ):
            ones2 = const_pool.tile([2, 128], f16)
            nc.vector.memset(ones2, 1.0)
            wu_mov = const_pool.tile([2, MBS], f16)
            nc.vector.memset(wu_mov, 1.0)
            bias_sb = const_pool.tile([128, PP * NT], f32)
            rowsums = const_pool.tile([128, PP * NT], f32)
            wu_exp = const_pool.tile([128, 16], bf16)

            # ---- input DMAs: all on the sync ring, priority order ----
            xt_sb = [
                xy_pool.tile([128, KT * N], f8, tag="xt", name=f"xt{j}")
                for j in range(PP)
            ]
            yt_sb = [
                xy_pool.tile([128, KT * N], f8, tag="yt", name=f"yt{j}")
                for j in range(PP)
            ]
            ybc_sb = [
                bc_pool.tile([128, N], f16, tag="ybc", name=f"ybc{j}")
                for j in range(PP)
            ]
            nc.sync.dma_start(out=ybc_sb[0], in_=ybc_d[0])
            for q in range(4):
                sl = slice(q * QRT, (q + 1) * QRT)
                nc.sync.dma_start(out=xt_sb[0][:, sl], in_=xt_d[0, :, sl])
                nc.sync.dma_start(out=yt_sb[0][:, sl], in_=yt_d[0, :, sl])
            nc.sync.dma_start(out=bias_sb, in_=bias_d)
            for j in (1, 2):
                nc.sync.dma_start(out=ybc_sb[j], in_=ybc_d[j])
                for hq in range(2):
                    sl = slice(hq * 2 * QRT, (hq + 1) * 2 * QRT)
                    nc.sync.dma_start(out=xt_sb[j][:, sl], in_=xt_d[j, :, sl])
                    nc.sync.dma_start(out=yt_sb[j][:, sl], in_=yt_d[j, :, sl])

            # ---- warm-ups while DMAs land: PE HAM -> 8/8, ACT exp table ----
            nc.scalar.activation(
                out=wu_exp, in_=wu_exp, func=EXP, bias=0.0, scale=-1.0
            )
            ps_wu = ps_pool.tile([128, MB, MBS], f32, tag="ps")
            for i in range(N_WARMUP):
                nc.tensor.matmul(
                    ps_wu[:, i % MB, :], ones2, wu_mov, start=True, stop=True
                )

            units = [(j, nt) for j in range(PP) for nt in range(NT)]
            colaccs = [None] * PP

            def preload(u):
                ps_u = ps_pool.tile([128, MB, MBS], f32, tag="ps", name=f"ps{u}")
                jv = units[u][0]
                src = ybc_sb[jv].rearrange("p (a m) -> p a m", a=MB)
                if u % 3 == 0:
                    nc.scalar.copy(out=ps_u, in_=src)
                else:
                    nc.vector.tensor_scalar(
                        out=ps_u, in0=src, scalar1=0.0, scalar2=None, op0=op_add
                    )
                return ps_u

            # preloads run TWO tiles ahead of the copy-outs: an ACT preload
            # plus the exp copy-out don't both fit in one PE tile-window
            # (~1.73us), so stagger depth 1 stalls the PE ~1.1us every
            # third tile; depth 2 gives ACT two windows of slack to catch up.
            ps_q = [preload(0), preload(1)]
            for u, (j, nt) in enumerate(units):
                ps = ps_q.pop(0)
                xt3 = xt_sb[j].rearrange("p (k n) -> p k n", k=KT)
                yt3 = yt_sb[j].rearrange("p (k n) -> p k n", k=KT)
                nsl = slice(nt * 128, (nt + 1) * 128)
                for ki in range(KT // 2):
                    xsl = xt3[:, 2 * ki : 2 * ki + 2, nsl]
                    for mb in range(MB):
                        nc.tensor.matmul(
                            ps[:, mb, :],
                            xsl,
                            yt3[:, 2 * ki : 2 * ki + 2, mb * MBS : (mb + 1) * MBS],
                            start=False,  # accumulate onto the y2bc preload
                            stop=(ki == KT // 2 - 1 and mb == MB - 1),
                            perf_mode=mybir.MatmulPerfMode.DoubleRow,
                        )
                if u + 2 < len(units):
                    ps_q.append(preload(u + 2))
                idx = j * NT + nt
                # ONE ACT op: E = exp((ref - x2[n] - psum)/T) -> bf16 SBUF,
                # fused rowsums[:, idx] = sum_m E
                ee = d2_pool.tile([128, N], bf16, tag="ee")
                nc.scalar.activation(
                    out=ee.rearrange("p (a m) -> p a m", a=MB),
                    in_=ps,
                    func=EXP,
                    bias=bias_sb[:, idx : idx + 1],
                    scale=-1.0 / T_SMOOTH,
                    accum_out=rowsums[:, idx : idx + 1],
                )
                # colacc = max(colacc, E)   (bf16 packed TT)
                if nt == 0:
                    colaccs[j] = col_pool.tile(
                        [128, N], bf16, tag="colacc", name=f"col{j}"
                    )
                    nc.vector.tensor_scalar(
                        out=colaccs[j],
                        in0=ee,
                        scalar1=0.0,
                        scalar2=None,
                        op0=op_add,
                    )
                else:
                    nc.vector.tensor_tensor(
                        out=colaccs[j], in0=ee, in1=colaccs[j], op=op_max
                    )
                if nt == NT - 1:
                    nc.sync.dma_start(out=col_d[j], in_=colaccs[j])
            nc.sync.dma_start(out=row_d, in_=rowsums)
    _legalize_sync(nc)
    return nc


def _prep_inputs(x, y):
    import ml_dtypes

    f8np = np.dtype(ml_dtypes.float8_e4m3)
    x32 = np.ascontiguousarray(x, dtype=np.float32).reshape(PAIRS, N, D)
    y32 = np.ascontiguousarray(y, dtype=np.float32).reshape(PAIRS, N, D)

    # xtp[q, p, k*N + n] = -2 x[q, n, k*128 + p]; ytp[q, p, k*N+m] = y[q,m,k*128+p]
    xtp = np.empty((PAIRS, 128, KT * N), f8np)
    ytp = np.empty((PAIRS, 128, KT * N), f8np)
    for q in range(PAIRS):
        xt = (x32[q].T * np.float32(-2.0)).astype(f8np)   # [D, N]
        yt = y32[q].T.astype(f8np)
        xtp[q] = xt.reshape(KT, 128, N).transpose(1, 0, 2).reshape(128, KT * N)
        ytp[q] = yt.reshape(KT, 128, N).transpose(1, 0, 2).reshape(128, KT * N)

    x2 = np.square(x32.astype(np.float64)).sum(-1)  # [PAIRS, N]
    y2 = np.square(y32.astype(np.float64)).sum(-1)
    # y2 broadcast to all partitions, fp16 (quantization +-0.5 at ~2048)
    y2bc = np.ascontiguousarray(
        np.broadcast_to(y2.astype(np.float16)[:, None, :], (PAIRS, 128, N))
    )
    # biasT[q, p, t] = (REF - x2[q, t*128+p]) / T
    biasT = np.ascontiguousarray(
        ((REF - x2) / T_SMOOTH)
        .reshape(PAIRS, NT, 128)
        .transpose(0, 2, 1)
        .astype(np.float32)
    )
    return xtp, ytp, y2bc, biasT


def _run(x, y, trace=False):
    global _NC_CACHE
    from concourse.bass_utils import run_bass_kernel_spmd

    xtp, ytp, y2bc, biasT = _prep_inputs(x, y)

    if _NC_CACHE is None:
        _NC_CACHE = _build_nc()
    nc = _NC_CACHE

    in_maps = []
    for i in range(N_CORES):
        q0 = i * PP
        bias_core = np.ascontiguousarray(
            biasT[q0 : q0 + PP].transpose(1, 0, 2).reshape(128, PP * NT)
        )
        in_maps.append(
            {
                "xtp": xtp[q0 : q0 + PP],
                "ytp": ytp[q0 : q0 + PP],
                "y2bc": y2bc[q0 : q0 + PP],
                "biasT": bias_core,
            }
        )

    res = run_bass_kernel_spmd(nc, in_maps, core_ids=list(range(N_CORES)), trace=trace)

    TINY = 1e-300
    h2 = np.empty(PAIRS, np.float64)
    for i in range(N_CORES):
        r = res.results[i]
        rs = r["rowsums"].astype(np.float64)  # [128, PP*NT]
        for j in range(PP):
            # fwd2 = max_n (ref - T ln rowsum[n]) = ref - T ln(min rowsum)
            rmin = max(rs[:, j * NT : (j + 1) * NT].min(), TINY)
            fwd2 = REF - T_SMOOTH * np.log(rmin)
            # bwd2 = max_m (ref - T ln(max_p colacc[p, m]))
            cmax = np.maximum(
                r["colout"][j].astype(np.float64).max(0), TINY
            )  # [N]
            bwd2 = REF - T_SMOOTH * np.log(cmax.min())
            h2[i * PP + j] = max(fwd2, bwd2, 0.0)

    ans = np.sqrt(h2).mean()
    return np.array(ans, dtype=np.float32), res


def kernel(input, target):
    out, _ = _run(np.asarray(input), np.asarray(target), trace=False)
    return out
